# revision 32
# baseline (speedup 1.0000x reference)
"""Multi-head attention forward on 8 Trainium2 NeuronCores.

Problem: B=32, N=512, C=1024, H=16 heads, head_dim=64, fp32 I/O.
Strategy: data-parallel over batch (4 batches per core), no collectives.

Math notes:
  - reference adds mask[:,None,None,:] + mask[:,None,:,None] to the logits;
    the query-axis term is constant along the softmax axis so it cancels.
    The key-axis term is folded in as exp(mask)[k], applied by scaling V rows
    and the softmax-denominator column.
  - softmax denominator comes out of the attn@V matmul for free: V is
    augmented with a 65th column holding exp(mask)[k], so
    out[q,64] = sum_k e[q,k]*em[k] = denominator.

Layouts (per core, per batch):
  xT [c,n] (host pre-transposed) -> Q^T,K^T [dc,n] via W-stationary matmuls,
  V [n,dc] via x-stationary matmuls (scaled by em on evac).
  scores^T [k,q] per head (contract d=64), exp on ScalarE -> e^T bf16.
  attn@V: lhsT=e^T[k,q-tile], rhs=V_aug[k,65] -> psum [q,65]; normalize with
  reciprocal of col 64 (per-partition scalar) -> attn [q,c] bf16.
  PE-transpose attn -> attn^T [c,q]; proj: lhsT=attn^T, rhs=W_proj^T -> out.
"""
import numpy as np
import ml_dtypes

B, N, C, H = 32, 512, 1024, 16
HD = C // H  # 64
SCALE = HD ** -0.5
NCORES = 8
BL = B // NCORES  # batches per core = 4
CT = C // 128     # 8 c-tiles
NT = N // 128     # 4 n-tiles
DC3 = 3 * C       # 3072

_cached_nc = None


def _build(repeat=1, bufs_big=4, bufs_av=2, bufs_tr=2):
    import contextlib
    import concourse.mybir as mybir
    import concourse.tile as tile
    from concourse import bacc
    from concourse.masks import make_identity

    BF16 = mybir.dt.bfloat16
    F32 = mybir.dt.float32
    EXP = mybir.ActivationFunctionType.Exp

    nc = bacc.Bacc()
    xT_d = nc.dram_tensor("xT", [BL, C, N], BF16, kind="ExternalInput")
    wqkvT_d = nc.dram_tensor("wqkvT", [C, DC3], BF16, kind="ExternalInput")
    wprojT_d = nc.dram_tensor("wprojT", [C, C], BF16, kind="ExternalInput")
    em_d = nc.dram_tensor("em", [128, BL, NT], F32, kind="ExternalInput")
    out_d = nc.dram_tensor("out", [BL, N, C], F32, kind="ExternalOutput")

    with tile.TileContext(nc) as tc:
        with (
            tc.tile_pool(name="singles", bufs=1) as singles,
            tc.tile_pool(name="xp", bufs=2) as xp,
            tc.tile_pool(name="qkp", bufs=2) as qkp,
            tc.tile_pool(name="vp", bufs=2) as vp,
            tc.tile_pool(name="ep", bufs=3) as ep,
            tc.tile_pool(name="ap", bufs=2) as ap,
            tc.tile_pool(name="atp", bufs=2) as atp,
            tc.tile_pool(name="op", bufs=3) as op,
            tc.tile_pool(name="rp", bufs=8) as rp,
            tc.tile_pool(name="ps_big", bufs=bufs_big, space="PSUM") as ps_big,
            tc.tile_pool(name="ps_av", bufs=bufs_av, space="PSUM") as ps_av,
            tc.tile_pool(name="ps_tr", bufs=bufs_tr, space="PSUM") as ps_tr,
        ):
            # --- one-time loads, ordered so the first QKV group unblocks
            # after ~1.25 MB: x^T(b=0) + the dct=0 weight slice. Weights are
            # chunked along the OUTPUT (dct) axis because each accumulation
            # group needs all 8 c-tiles of its slice but nothing else. ---
            em_sb = singles.tile([128, BL, NT], F32)
            nc.sync.dma_start(out=em_sb[:], in_=em_d[:])
            xT_first = xp.tile([128, CT, N], BF16, tag="xT")
            nc.sync.dma_start(out=xT_first[:],
                              in_=xT_d[0].rearrange("(ct p) n -> p ct n", p=128))
            wqkvT_sb = singles.tile([128, CT, DC3], BF16)
            wqkvT_src = wqkvT_d.rearrange("(ct p) d -> p ct d", p=128)
            for j in range(24):
                nc.sync.dma_start(
                    out=wqkvT_sb[:, :, j * 128:(j + 1) * 128],
                    in_=wqkvT_src[:, :, j * 128:(j + 1) * 128])
            ident = singles.tile([128, 128], BF16)
            make_identity(nc, ident[:])
            wprojT_sb = singles.tile([128, CT, C], BF16)
            nc.sync.dma_start(out=wprojT_sb[:],
                              in_=wprojT_d.rearrange("(ct p) d -> p ct d", p=128))

            rep_ctx = tc.For_i(0, repeat, 1) if repeat > 1 else contextlib.nullcontext()
            with rep_ctx:
              for b in range(BL):
                # --- load x^T for this batch (b=0 preloaded above) ---
                if b == 0:
                    xT_sb = xT_first
                else:
                    xT_sb = xp.tile([128, CT, N], BF16, tag="xT")
                    nc.sync.dma_start(
                        out=xT_sb[:],
                        in_=xT_d[b].rearrange("(ct p) n -> p ct n", p=128))

                # --- Q^T, K^T: [dc,n] tiles, dc-tiles 0..7=Q, 8..15=K ---
                qkT_sb = qkp.tile([128, 16, N], BF16, tag="qkT")
                for dct in range(16):
                    ps = ps_big.tile([128, 512], F32, tag="big")
                    for ct in range(CT):
                        nc.tensor.matmul(
                            ps[:],
                            wqkvT_sb[:, ct, dct * 128:(dct + 1) * 128],
                            xT_sb[:, ct, :],
                            start=(ct == 0), stop=(ct == CT - 1))
                    nc.vector.tensor_copy(out=qkT_sb[:, dct, :], in_=ps[:])

                # --- V natural [n, dc_v], scaled by em, + aug col ---
                vaug_sb = vp.tile([128, NT, H, HD + 1], BF16, tag="vaug")
                for nt in range(NT):
                    for dcv in range(2):
                        ps = ps_big.tile([128, 512], F32, tag="big")
                        for ct in range(CT):
                            nc.tensor.matmul(
                                ps[:],
                                xT_sb[:, ct, nt * 128:(nt + 1) * 128],
                                wqkvT_sb[:, ct, 2 * C + dcv * 512:2 * C + (dcv + 1) * 512],
                                start=(ct == 0), stop=(ct == CT - 1))
                        nc.vector.tensor_scalar_mul(
                            vaug_sb[:, nt, 8 * dcv:8 * (dcv + 1), 0:HD],
                            ps.rearrange("p (h d) -> p h d", d=HD),
                            em_sb[:, b, nt:nt + 1])
                    nc.vector.tensor_copy(
                        out=vaug_sb[:, nt, :, HD],
                        in_=em_sb[:, b, nt:nt + 1].to_broadcast((128, H)))

                # --- per head: scores^T + exp, then attn@V_aug + normalize.
                # Software-pipelined: scores for head h+1 are emitted before
                # attn@V of head h so PE does not stall on ScalarE's exp. ---
                attn_sb = ap.tile([128, NT, C], BF16, tag="attn")
                eT_tiles = {}

                def emit_scores(h):
                    dct_q = h // 2
                    dct_k = 8 + h // 2
                    po = (h % 2) * HD
                    eT_sb = ep.tile([128, NT, N], BF16, tag="eT")
                    for kt in range(NT):
                        ps = ps_big.tile([128, 512], F32, tag="big")
                        nc.tensor.matmul(
                            ps[:],
                            qkT_sb[po:po + HD, dct_k, kt * 128:(kt + 1) * 128],
                            qkT_sb[po:po + HD, dct_q, :],
                            start=True, stop=True)
                        nc.scalar.activation(eT_sb[:, kt, :], ps[:], EXP, scale=SCALE)
                    eT_tiles[h] = eT_sb

                def emit_attnv(h):
                    eT_sb = eT_tiles.pop(h)
                    for qt in range(NT):
                        psa = ps_av.tile([128, HD + 1], F32, tag="av")
                        for kt in range(NT):
                            nc.tensor.matmul(
                                psa[:],
                                eT_sb[:, kt, qt * 128:(qt + 1) * 128],
                                vaug_sb[:, kt, h, :],
                                start=(kt == 0), stop=(kt == NT - 1))
                        recip = rp.tile([128, 1], F32, tag="recip")
                        nc.vector.reciprocal(recip[:], psa[:, HD:HD + 1])
                        nc.vector.tensor_scalar_mul(
                            attn_sb[:, qt, h * HD:(h + 1) * HD],
                            psa[:, 0:HD], recip[:])

                emit_scores(0)
                for h in range(H):
                    if h + 1 < H:
                        emit_scores(h + 1)
                    emit_attnv(h)

                # --- transpose attn -> attn^T [c, q] ---
                attnT_sb = atp.tile([128, CT, N], BF16, tag="attnT")
                for qt in range(NT):
                    for ct in range(CT):
                        pst = ps_tr.tile([128, 128], BF16, tag="tr")
                        nc.tensor.transpose(
                            pst[:], attn_sb[:, qt, ct * 128:(ct + 1) * 128], ident[:])
                        nc.vector.tensor_copy(
                            out=attnT_sb[:, ct, qt * 128:(qt + 1) * 128], in_=pst[:])

                # --- projection ---
                for qt in range(NT):
                    out_sb = op.tile([128, C], F32, tag="out")
                    for cot in range(2):
                        ps = ps_big.tile([128, 512], F32, tag="big")
                        for ct in range(CT):
                            nc.tensor.matmul(
                                ps[:],
                                attnT_sb[:, ct, qt * 128:(qt + 1) * 128],
                                wprojT_sb[:, ct, cot * 512:(cot + 1) * 512],
                                start=(ct == 0), stop=(ct == CT - 1))
                        nc.vector.tensor_copy(out=out_sb[:, cot * 512:(cot + 1) * 512], in_=ps[:])
                    nc.sync.dma_start(out=out_d[b, qt * 128:(qt + 1) * 128, :],
                                      in_=out_sb[:])
    nc.finalize()
    return nc


def _prep_inputs(x, mask, W_qkv, W_proj):
    bf16 = ml_dtypes.bfloat16
    xT = np.ascontiguousarray(x.transpose(0, 2, 1)).astype(bf16)      # [B, C, N]
    wqkvT = np.ascontiguousarray(W_qkv.T).astype(bf16)                # [C, 3C]
    wprojT = np.ascontiguousarray(W_proj.T).astype(bf16)              # [C, C]
    em = np.exp(mask).astype(np.float32)                              # [B, N]
    # pre-striped for SBUF layout [p, b_local, nt]: em[b, nt*128+p]
    em_striped = np.ascontiguousarray(
        em.reshape(B, NT, 128).transpose(2, 0, 1))        # [128, B, NT]
    return [
        {
            "xT": xT[c * BL:(c + 1) * BL],
            "wqkvT": wqkvT,
            "wprojT": wprojT,
            "em": em_striped[:, c * BL:(c + 1) * BL, :],
        }
        for c in range(NCORES)
    ]


def build_null():
    """Null kernel with the same external I/O — timing baseline."""
    import concourse.mybir as mybir
    import concourse.tile as tile
    from concourse import bacc

    BF16 = mybir.dt.bfloat16
    F32 = mybir.dt.float32
    nc = bacc.Bacc()
    nc.dram_tensor("xT", [BL, C, N], BF16, kind="ExternalInput")
    nc.dram_tensor("wqkvT", [C, DC3], BF16, kind="ExternalInput")
    nc.dram_tensor("wprojT", [C, C], BF16, kind="ExternalInput")
    em_d = nc.dram_tensor("em", [128, BL, NT], F32, kind="ExternalInput")
    out_d = nc.dram_tensor("out", [BL, N, C], F32, kind="ExternalOutput")
    with tile.TileContext(nc) as tc:
        with tc.tile_pool(name="sb", bufs=1) as sb:
            t = sb.tile([128, 4], F32)
            nc.sync.dma_start(out=t[:], in_=em_d[:, 0, :])
            nc.sync.dma_start(out=out_d[0, 0:128, 0:4], in_=t[:])
    nc.finalize()
    return nc


def get_nc():
    global _cached_nc
    if _cached_nc is None:
        _cached_nc = _build()
    return _cached_nc


def kernel(x, mask, W_qkv, W_proj, b_proj):
    from concourse.bass_utils import run_bass_kernel_spmd

    nc = get_nc()
    in_maps = _prep_inputs(np.asarray(x, dtype=np.float32),
                           np.asarray(mask, dtype=np.float32),
                           np.asarray(W_qkv, dtype=np.float32),
                           np.asarray(W_proj, dtype=np.float32))
    res = run_bass_kernel_spmd(nc, in_maps, core_ids=list(range(NCORES)))
    out = np.concatenate([res.results[c]["out"] for c in range(NCORES)], axis=0)
    out = out + np.asarray(b_proj, dtype=np.float32)[None, None, :]
    return np.ascontiguousarray(out.astype(np.float32))


# revision 34
# speedup vs baseline: 1.6460x; 1.6460x over previous
"""Multi-head attention forward on 8 Trainium2 NeuronCores.

Problem: B=32, N=512, C=1024, H=16 heads, head_dim=64, fp32 I/O.
Strategy: data-parallel over batch (4 batches per core), no collectives.

Math notes:
  - reference adds mask[:,None,None,:] + mask[:,None,:,None] to the logits;
    the query-axis term is constant along the softmax axis so it cancels.
    The key-axis term is folded in as exp(mask)[k], applied by scaling V rows
    and the softmax-denominator column.
  - softmax denominator comes out of the attn@V matmul for free: V is
    augmented with a 65th column holding exp(mask)[k], so
    out[q,64] = sum_k e[q,k]*em[k] = denominator.

Layouts (per core, per batch):
  xT [c,n] (host pre-transposed) -> Q^T,K^T [dc,n] via W-stationary matmuls,
  V [n,dc] via x-stationary matmuls (scaled by em on evac).
  scores^T [k,q] per head (contract d=64), exp on ScalarE -> e^T bf16.
  attn@V: lhsT=e^T[k,q-tile], rhs=V_aug[k,65] -> psum [q,65]; normalize with
  reciprocal of col 64 (per-partition scalar) -> attn [q,c] bf16.
  PE-transpose attn -> attn^T [c,q]; proj: lhsT=attn^T, rhs=W_proj^T -> out.
"""
import numpy as np
import ml_dtypes

B, N, C, H = 32, 512, 1024, 16
HD = C // H  # 64
SCALE = HD ** -0.5
NCORES = 8
BL = B // NCORES  # batches per core = 4
CT = C // 128     # 8 c-tiles
NT = N // 128     # 4 n-tiles
DC3 = 3 * C       # 3072

_cached_nc = None


def _build(repeat=1, bufs_big=4, bufs_av=2, bufs_tr=2):
    import contextlib
    import concourse.mybir as mybir
    import concourse.tile as tile
    from concourse import bacc
    from concourse.masks import make_identity

    BF16 = mybir.dt.bfloat16
    F32 = mybir.dt.float32
    EXP = mybir.ActivationFunctionType.Exp

    nc = bacc.Bacc()
    xT_d = nc.dram_tensor("xT", [BL, C, N], BF16, kind="ExternalInput")
    wqkvT_d = nc.dram_tensor("wqkvT", [C, DC3], BF16, kind="ExternalInput")
    wprojT_d = nc.dram_tensor("wprojT", [C, C], BF16, kind="ExternalInput")
    em_d = nc.dram_tensor("em", [128, BL, NT], F32, kind="ExternalInput")
    out_d = nc.dram_tensor("out", [BL, N, C], F32, kind="ExternalOutput")

    with tile.TileContext(nc) as tc:
        with (
            tc.tile_pool(name="singles", bufs=1) as singles,
            tc.tile_pool(name="xp", bufs=2) as xp,
            tc.tile_pool(name="qkp", bufs=2) as qkp,
            tc.tile_pool(name="vp", bufs=2) as vp,
            tc.tile_pool(name="ep", bufs=3) as ep,
            tc.tile_pool(name="ap", bufs=2) as ap,
            tc.tile_pool(name="atp", bufs=2) as atp,
            tc.tile_pool(name="op", bufs=3) as op,
            tc.tile_pool(name="rp", bufs=8) as rp,
            tc.tile_pool(name="ps_big", bufs=bufs_big, space="PSUM") as ps_big,
            tc.tile_pool(name="ps_av", bufs=bufs_av, space="PSUM") as ps_av,
            tc.tile_pool(name="ps_tr", bufs=bufs_tr, space="PSUM") as ps_tr,
        ):
            # --- one-time loads, ordered so the first QKV group unblocks
            # after ~1.25 MB: x^T(b=0) + the dct=0 weight slice. Weights are
            # chunked along the OUTPUT (dct) axis because each accumulation
            # group needs all 8 c-tiles of its slice but nothing else. ---
            em_sb = singles.tile([128, BL, NT], F32)
            nc.sync.dma_start(out=em_sb[:], in_=em_d[:])
            xT_first = xp.tile([128, CT, N], BF16, tag="xT")
            nc.sync.dma_start(out=xT_first[:],
                              in_=xT_d[0].rearrange("(ct p) n -> p ct n", p=128))
            wqkvT_sb = singles.tile([128, CT, DC3], BF16)
            wqkvT_src = wqkvT_d.rearrange("(ct p) d -> p ct d", p=128)
            for j in range(24):
                nc.sync.dma_start(
                    out=wqkvT_sb[:, :, j * 128:(j + 1) * 128],
                    in_=wqkvT_src[:, :, j * 128:(j + 1) * 128])
            ident = singles.tile([128, 128], BF16)
            make_identity(nc, ident[:])
            wprojT_sb = singles.tile([128, CT, C], BF16)
            nc.sync.dma_start(out=wprojT_sb[:],
                              in_=wprojT_d.rearrange("(ct p) d -> p ct d", p=128))

            rep_ctx = tc.For_i(0, repeat, 1) if repeat > 1 else contextlib.nullcontext()
            with rep_ctx:
              for b in range(BL):
                # --- load x^T for this batch (b=0 preloaded above) ---
                if b == 0:
                    xT_sb = xT_first
                else:
                    xT_sb = xp.tile([128, CT, N], BF16, tag="xT")
                    nc.sync.dma_start(
                        out=xT_sb[:],
                        in_=xT_d[b].rearrange("(ct p) n -> p ct n", p=128))

                # --- Q^T, K^T: [dc,n] tiles, dc-tiles 0..7=Q, 8..15=K ---
                qkT_sb = qkp.tile([128, 16, N], BF16, tag="qkT")
                for dct in range(16):
                    ps = ps_big.tile([128, 512], F32, tag="big")
                    for ct in range(CT):
                        nc.tensor.matmul(
                            ps[:],
                            wqkvT_sb[:, ct, dct * 128:(dct + 1) * 128],
                            xT_sb[:, ct, :],
                            start=(ct == 0), stop=(ct == CT - 1))
                    nc.vector.tensor_copy(out=qkT_sb[:, dct, :], in_=ps[:])

                # --- V natural [n, dc_v], scaled by em, + aug col ---
                vaug_sb = vp.tile([128, NT, H, HD + 1], BF16, tag="vaug")
                for nt in range(NT):
                    for dcv in range(2):
                        ps = ps_big.tile([128, 512], F32, tag="big")
                        for ct in range(CT):
                            nc.tensor.matmul(
                                ps[:],
                                xT_sb[:, ct, nt * 128:(nt + 1) * 128],
                                wqkvT_sb[:, ct, 2 * C + dcv * 512:2 * C + (dcv + 1) * 512],
                                start=(ct == 0), stop=(ct == CT - 1))
                        nc.vector.tensor_scalar_mul(
                            vaug_sb[:, nt, 8 * dcv:8 * (dcv + 1), 0:HD],
                            ps.rearrange("p (h d) -> p h d", d=HD),
                            em_sb[:, b, nt:nt + 1])
                    nc.vector.tensor_copy(
                        out=vaug_sb[:, nt, :, HD],
                        in_=em_sb[:, b, nt:nt + 1].to_broadcast((128, H)))

                # --- per head: scores^T + exp, then attn@V_aug + normalize.
                # Software-pipelined: scores for head h+1 are emitted before
                # attn@V of head h so PE does not stall on ScalarE's exp. ---
                attn_sb = ap.tile([128, NT, C], BF16, tag="attn")
                eT_tiles = {}

                def emit_scores(h):
                    dct_q = h // 2
                    dct_k = 8 + h // 2
                    po = (h % 2) * HD
                    eT_sb = ep.tile([128, NT, N], BF16, tag="eT")
                    for kt in range(NT):
                        ps = ps_big.tile([128, 512], F32, tag="big")
                        nc.tensor.matmul(
                            ps[:],
                            qkT_sb[po:po + HD, dct_k, kt * 128:(kt + 1) * 128],
                            qkT_sb[po:po + HD, dct_q, :],
                            start=True, stop=True)
                        nc.scalar.activation(eT_sb[:, kt, :], ps[:], EXP, scale=SCALE)
                    eT_tiles[h] = eT_sb

                def emit_attnv(h):
                    eT_sb = eT_tiles.pop(h)
                    for qt in range(NT):
                        psa = ps_av.tile([128, HD + 1], F32, tag="av")
                        for kt in range(NT):
                            nc.tensor.matmul(
                                psa[:],
                                eT_sb[:, kt, qt * 128:(qt + 1) * 128],
                                vaug_sb[:, kt, h, :],
                                start=(kt == 0), stop=(kt == NT - 1))
                        recip = rp.tile([128, 1], F32, tag="recip")
                        nc.vector.reciprocal(recip[:], psa[:, HD:HD + 1])
                        nc.vector.tensor_scalar_mul(
                            attn_sb[:, qt, h * HD:(h + 1) * HD],
                            psa[:, 0:HD], recip[:])

                emit_scores(0)
                for h in range(H):
                    if h + 1 < H:
                        emit_scores(h + 1)
                    emit_attnv(h)

                # --- transpose attn -> attn^T [c, q] ---
                attnT_sb = atp.tile([128, CT, N], BF16, tag="attnT")
                for qt in range(NT):
                    for ct in range(CT):
                        pst = ps_tr.tile([128, 128], BF16, tag="tr")
                        nc.tensor.transpose(
                            pst[:], attn_sb[:, qt, ct * 128:(ct + 1) * 128], ident[:])
                        nc.vector.tensor_copy(
                            out=attnT_sb[:, ct, qt * 128:(qt + 1) * 128], in_=pst[:])

                # --- projection ---
                for qt in range(NT):
                    out_sb = op.tile([128, C], F32, tag="out")
                    for cot in range(2):
                        ps = ps_big.tile([128, 512], F32, tag="big")
                        for ct in range(CT):
                            nc.tensor.matmul(
                                ps[:],
                                attnT_sb[:, ct, qt * 128:(qt + 1) * 128],
                                wprojT_sb[:, ct, cot * 512:(cot + 1) * 512],
                                start=(ct == 0), stop=(ct == CT - 1))
                        nc.vector.tensor_copy(out=out_sb[:, cot * 512:(cot + 1) * 512], in_=ps[:])
                    nc.sync.dma_start(out=out_d[b, qt * 128:(qt + 1) * 128, :],
                                      in_=out_sb[:])
    nc.finalize()
    return nc


def _prep_inputs(x, mask, W_qkv, W_proj):
    bf16 = ml_dtypes.bfloat16
    xT = np.ascontiguousarray(x.transpose(0, 2, 1)).astype(bf16)      # [B, C, N]
    wqkvT = np.ascontiguousarray(W_qkv.T).astype(bf16)                # [C, 3C]
    wprojT = np.ascontiguousarray(W_proj.T).astype(bf16)              # [C, C]
    em = np.exp(mask).astype(np.float32)                              # [B, N]
    # pre-striped for SBUF layout [p, b_local, nt]: em[b, nt*128+p]
    em_striped = np.ascontiguousarray(
        em.reshape(B, NT, 128).transpose(2, 0, 1))        # [128, B, NT]
    return [
        {
            "xT": xT[c * BL:(c + 1) * BL],
            "wqkvT": wqkvT,
            "wprojT": wprojT,
            "em": em_striped[:, c * BL:(c + 1) * BL, :],
        }
        for c in range(NCORES)
    ]


def build_null():
    """Null kernel with the same external I/O — timing baseline."""
    import concourse.mybir as mybir
    import concourse.tile as tile
    from concourse import bacc

    BF16 = mybir.dt.bfloat16
    F32 = mybir.dt.float32
    nc = bacc.Bacc()
    nc.dram_tensor("xT", [BL, C, N], BF16, kind="ExternalInput")
    nc.dram_tensor("wqkvT", [C, DC3], BF16, kind="ExternalInput")
    nc.dram_tensor("wprojT", [C, C], BF16, kind="ExternalInput")
    em_d = nc.dram_tensor("em", [128, BL, NT], F32, kind="ExternalInput")
    out_d = nc.dram_tensor("out", [BL, N, C], F32, kind="ExternalOutput")
    with tile.TileContext(nc) as tc:
        with tc.tile_pool(name="sb", bufs=1) as sb:
            t = sb.tile([128, 4], F32)
            nc.sync.dma_start(out=t[:], in_=em_d[:, 0, :])
            nc.sync.dma_start(out=out_d[0, 0:128, 0:4], in_=t[:])
    nc.finalize()
    return nc


def get_nc():
    global _cached_nc
    if _cached_nc is None:
        _cached_nc = _build()
    return _cached_nc


def kernel(x, mask, W_qkv, W_proj, b_proj):
    from concourse.bass_utils import run_bass_kernel_spmd

    nc = get_nc()
    in_maps = _prep_inputs(np.asarray(x, dtype=np.float32),
                           np.asarray(mask, dtype=np.float32),
                           np.asarray(W_qkv, dtype=np.float32),
                           np.asarray(W_proj, dtype=np.float32))
    res = run_bass_kernel_spmd(nc, in_maps, core_ids=list(range(NCORES)))
    out = np.concatenate([res.results[c]["out"] for c in range(NCORES)], axis=0)
    out = out + np.asarray(b_proj, dtype=np.float32)[None, None, :]
    return np.ascontiguousarray(out.astype(np.float32))
